# revision 2
# baseline (speedup 1.0000x reference)
"""Llama attention (b=2, s=2048, h=4096, 32 q-heads / 8 kv-heads, d=128) on 8
Trainium2 NeuronCores.

Sharding: 2-way data parallel over batch x 4-way tensor parallel over head
groups. Core c handles batch c//4 and q-heads [8*(c%4), 8*(c%4)+8) (kv-heads
[2*(c%4), 2*(c%4)+2)). Each core computes a partial o_proj output (contraction
over its 1024 head dims); the host sums the 4 partials per batch and adds bo.

Device kernel (per core), all matmuls in float32r (fp22 mantissa, full PE rate):
  P1  QT/KT = W^T @ X^T accumulated over the 4096 contraction (RoPE fused into
      the PSUM eviction), V = X @ Wv in natural [s, d] layout (XT used as lhsT).
  P2  Scores are computed TRANSPOSED: ST[k, q] = K @ Q^T, so the mask bias and
      1/sqrt(32) scale fuse into the Exp activation (per-partition bias), the
      softmax needs no max subtraction (|scaled scores| << 80), no transposes
      are needed for P@V (lhsT=V[k,d], rhs=expST[k,q]), and the softmax
      denominator is a ones-lhsT matmul accumulated on the PE.
  P3  out = attn_out @ Wo_shard as a plain GEMM from the [d, head, q] staging.
"""

import math
import sys

sys.path.insert(0, "/opt/pypackages")
sys.path.insert(0, "/opt/trn_rl_repo")

import numpy as np

_H, _NH, _NKV, _D = 4096, 32, 8, 128
_B, _S = 2, 2048
_NCORES = 8
_TP = 4              # head-group parallel factor (x2 batch parallel)
_QH = _NH // _TP     # 8 q heads per core
_KVH = _NKV // _TP   # 2 kv heads per core
_G = _NH // _NKV     # 4 q heads per kv head
_SCALE = 1.0 / math.sqrt(float(_NH))
_ROPE_BASE = 10000.0
_MASK_NEG = -1.0e5   # exp(-1e5) == 0.0 in fp32; avoids inf/nan of finfo.min

_built = {}


def build_module(S, H, QH, KVH, G, has_mask):
    """Build the per-core Bass module. Returns the compiled Bacc object."""
    from concourse import bacc
    import concourse.mybir as mybir
    import concourse.tile as tile

    FP = mybir.dt.float32
    FPR = mybir.dt.float32r
    Exp = mybir.ActivationFunctionType.Copy  # placeholder; set below
    Exp = mybir.ActivationFunctionType.Exp
    Copy = mybir.ActivationFunctionType.Copy

    KO = H // 128          # contraction tiles for projections
    KT = S // 128          # key tiles for attention
    NQB = S // 512         # 512-wide q blocks in P1
    NCH = 2 * NQB          # 256-wide xt chunks
    NQC = S // 512         # 512-wide q chunks in P2
    NT = H // 512          # 512-wide n chunks in P3
    NST = S // 128         # s tiles in P3
    NKV = KVH * 128        # v projection free size

    nc = bacc.Bacc("TRN2", target_bir_lowering=False, debug=False,
                   num_devices=_NCORES)

    xt = nc.dram_tensor("xt", [128, NCH, KO, 256], FPR, kind="ExternalInput")
    wq = nc.dram_tensor("wq", [128, QH, KO, 128], FPR, kind="ExternalInput")
    wk = nc.dram_tensor("wk", [128, KVH, KO, 128], FPR, kind="ExternalInput")
    wv = nc.dram_tensor("wv", [128, KO, NKV], FPR, kind="ExternalInput")
    wo = nc.dram_tensor("wo", [128, QH, H], FPR, kind="ExternalInput")
    cos = nc.dram_tensor("cos", [128, S], FP, kind="ExternalInput")
    sin = nc.dram_tensor("sin", [128, S], FP, kind="ExternalInput")
    if has_mask:
        maskb = nc.dram_tensor("maskb", [128, KT], FP, kind="ExternalInput")
    out = nc.dram_tensor("out", [S, H], FP, kind="ExternalOutput")

    def mm(ps, lhsT, rhs, start, stop):
        nc.tensor.matmul(ps, lhsT=lhsT, rhs=rhs, start=start, stop=stop)

    with tile.TileContext(nc) as tc:
        with tc.tile_pool(name="dram", bufs=1, space="DRAM") as dram:
            qt_d = dram.tile([QH, 128, S], FPR)
            kt_d = dram.tile([KVH, 128, S], FPR)
            v_d = dram.tile([KVH, S, 128], FPR)
            ot_d = dram.tile([QH, 128, S], FPR)

            # ---------------- P1: projections + RoPE ----------------
            with tc.tile_pool(name="p1c", bufs=1) as cpool, \
                 tc.tile_pool(name="p1xt", bufs=2) as xt_pool, \
                 tc.tile_pool(name="p1w", bufs=2) as w_pool, \
                 tc.tile_pool(name="p1ps", bufs=4, space="PSUM") as ps1, \
                 tc.tile_pool(name="p1pv", bufs=2, space="PSUM") as psv, \
                 tc.tile_pool(name="p1t", bufs=2) as tpool, \
                 tc.tile_pool(name="p1o", bufs=2) as opool:

                cos_sb = cpool.tile([128, S], FP)
                sin_sb = cpool.tile([128, S], FP)
                wv_sb = cpool.tile([128, KO, NKV], FPR)
                nc.sync.dma_start(cos_sb[:], cos[:])
                nc.sync.dma_start(sin_sb[:], sin[:])
                nc.sync.dma_start(wv_sb[:], wv[:])

                for qb in range(NQB):
                    xt_h = []
                    for hf in range(2):
                        t = xt_pool.tile([128, KO, 256], FPR, tag="xt")
                        nc.sync.dma_start(t[:], xt[:, 2 * qb + hf])
                        xt_h.append(t)
                    cs = slice(qb * 512, qb * 512 + 512)

                    for ih in range(QH + KVH):
                        is_q = ih < QH
                        w_sb = w_pool.tile([128, KO, 128], FPR, tag="w")
                        src = wq[:, ih] if is_q else wk[:, ih - QH]
                        nc.sync.dma_start(w_sb[:], src)
                        ps = ps1.tile([128, 512], FP)
                        for hf in range(2):
                            for ko in range(KO):
                                mm(ps[:, hf * 256:(hf + 1) * 256],
                                   w_sb[:, ko], xt_h[hf][:, ko],
                                   ko == 0, ko == KO - 1)
                        # RoPE fused eviction:
                        #  ro[0:64]   = p[0:64]*cos  + p[64:128]*(-sin_lo)
                        #  ro[64:128] = p[64:128]*cos + p[0:64]*(+sin_hi)
                        # (sin_sb rows 0:64 hold -sin, rows 64:128 hold +sin)
                        t1 = tpool.tile([128, 512], FP, tag="t1")
                        t2 = tpool.tile([128, 512], FP, tag="t2")
                        ro = opool.tile([128, 512], FPR, tag="ro")
                        nc.vector.tensor_mul(t1[:], ps[:], cos_sb[:, cs])
                        nc.vector.tensor_mul(t2[0:64], ps[64:128],
                                             sin_sb[0:64, cs])
                        nc.vector.tensor_mul(t2[64:128], ps[0:64],
                                             sin_sb[64:128, cs])
                        nc.vector.tensor_add(ro[:], t1[:], t2[:])
                        dst = qt_d[ih, :, cs] if is_q else kt_d[ih - QH, :, cs]
                        nc.sync.dma_start(dst, ro[:])

                    # V in natural [s, d] layout: lhsT = XT s-tile columns
                    for stl in range(4):
                        hf, c0 = stl // 2, (stl % 2) * 128
                        pv = psv.tile([128, NKV], FP)
                        for ko in range(KO):
                            mm(pv[:], xt_h[hf][:, ko, c0:c0 + 128],
                               wv_sb[:, ko], ko == 0, ko == KO - 1)
                        vo = opool.tile([128, NKV], FPR, tag="vo")
                        nc.scalar.activation(vo[:], pv[:], Copy)
                        s0 = qb * 512 + stl * 128
                        for kv in range(KVH):
                            nc.sync.dma_start(
                                v_d[kv, s0:s0 + 128, :],
                                vo[:, kv * 128:(kv + 1) * 128])

            # ---------------- P2: attention ----------------
            with tc.tile_pool(name="p2c", bufs=1) as cpool, \
                 tc.tile_pool(name="p2qt", bufs=2) as qt_pool, \
                 tc.tile_pool(name="p2e", bufs=6) as e_pool, \
                 tc.tile_pool(name="p2ot", bufs=2) as ot_pool, \
                 tc.tile_pool(name="p2sm", bufs=3) as sm_pool, \
                 tc.tile_pool(name="p2ps", bufs=3, space="PSUM") as s_psum, \
                 tc.tile_pool(name="p2po", bufs=2, space="PSUM") as o_psum, \
                 tc.tile_pool(name="p2pe", bufs=2, space="PSUM") as se_psum:

                kt_sb = cpool.tile([128, KVH, S], FPR)
                v_sb = cpool.tile([128, KVH, KT, 128], FPR)
                ones_f = cpool.tile([128, 1], FP)
                ones = cpool.tile([128, 1], FPR)
                nc.sync.dma_start(kt_sb[:], kt_d[:].rearrange("k p q -> p k q"))
                nc.sync.dma_start(
                    v_sb[:], v_d[:].rearrange("k (t p) d -> p k t d", p=128))
                nc.vector.memset(ones_f[:], 1.0)
                nc.vector.tensor_copy(ones[:], ones_f[:])
                if has_mask:
                    mb_sb = cpool.tile([128, KT], FP)
                    nc.sync.dma_start(mb_sb[:], maskb[:])

                for qc in range(NQC):
                    ccs = slice(qc * 512, qc * 512 + 512)
                    qt_sb = qt_pool.tile([128, QH, 512], FPR, tag="qt")
                    nc.sync.dma_start(
                        qt_sb[:], qt_d[:, :, ccs].rearrange("h p q -> p h q"))
                    ot_sb = ot_pool.tile([128, QH, 512], FPR, tag="ot")

                    for h in range(QH):
                        kv = h // G
                        o_ps = o_psum.tile([128, 512], FP)
                        se_ps = se_psum.tile([1, 512], FP)
                        for kt in range(KT):
                            s_ps = s_psum.tile([128, 512], FP)
                            mm(s_ps[:], kt_sb[:, kv, kt * 128:(kt + 1) * 128],
                               qt_sb[:, h], True, True)
                            e_sb = e_pool.tile([128, 512], FPR, tag="e")
                            if has_mask:
                                nc.scalar.activation(e_sb[:], s_ps[:], Exp,
                                                     bias=mb_sb[:, kt:kt + 1],
                                                     scale=_SCALE)
                            else:
                                nc.scalar.activation(e_sb[:], s_ps[:], Exp,
                                                     scale=_SCALE)
                            mm(o_ps[:], v_sb[:, kv, kt], e_sb[:],
                               kt == 0, kt == KT - 1)
                            mm(se_ps[:], ones[:], e_sb[:],
                               kt == 0, kt == KT - 1)
                        recip = sm_pool.tile([1, 512], FP, tag="rc")
                        nc.vector.reciprocal(recip[:], se_ps[:])
                        bc = sm_pool.tile([128, 512], FP, tag="bc")
                        nc.gpsimd.partition_broadcast(bc[:], recip[:])
                        nc.vector.tensor_mul(ot_sb[:, h], o_ps[:], bc[:])

                    nc.sync.dma_start(
                        ot_d[:, :, ccs].rearrange("h p q -> p h q"), ot_sb[:])

            # ---------------- P3: output projection ----------------
            with tc.tile_pool(name="p3c", bufs=1) as cpool, \
                 tc.tile_pool(name="p3w", bufs=2) as wo_pool, \
                 tc.tile_pool(name="p3o", bufs=3) as oo_pool, \
                 tc.tile_pool(name="p3ps", bufs=4, space="PSUM") as o3_psum:

                ot_full = cpool.tile([128, QH, S], FPR)
                nc.sync.dma_start(ot_full[:],
                                  ot_d[:].rearrange("h p q -> p h q"))
                for ncn in range(NT):
                    ncs = slice(ncn * 512, ncn * 512 + 512)
                    wo_sb = wo_pool.tile([128, QH, 512], FPR, tag="wo")
                    nc.sync.dma_start(wo_sb[:], wo[:, :, ncs])
                    for st in range(NST):
                        ps = o3_psum.tile([128, 512], FP)
                        for ho in range(QH):
                            mm(ps[:], ot_full[:, ho, st * 128:(st + 1) * 128],
                               wo_sb[:, ho], ho == 0, ho == QH - 1)
                        ob = oo_pool.tile([128, 512], FP, tag="ob")
                        nc.scalar.activation(ob[:], ps[:], Copy)
                        nc.sync.dma_start(out[st * 128:(st + 1) * 128, ncs],
                                          ob[:])

    nc.compile()
    return nc


def _get_module(has_mask):
    key = (_S, _H, _QH, _KVH, has_mask)
    if key not in _built:
        _built[key] = build_module(_S, _H, _QH, _KVH, _G, has_mask)
    return _built[key]


def make_core_inputs(hidden_states, attn_mask, position_ids, Wq, Wk, Wv, Wo,
                     core, has_mask, S=_S, H=_H, QH=_QH, KVH=_KVH):
    """Host-side shard + layout packing for one core."""
    bi, hg = core // _TP, core % _TP
    KO = H // 128
    NCH = 2 * (S // 512)
    f32 = np.float32

    XT = np.ascontiguousarray(hidden_states[bi].T.astype(f32))      # [H, S]
    xt = XT.reshape(KO, 128, NCH, 256).transpose(1, 2, 0, 3).copy()

    qcols = slice(hg * QH * 128, (hg + 1) * QH * 128)
    kvcols = slice(hg * KVH * 128, (hg + 1) * KVH * 128)
    wq = Wq[:, qcols].reshape(KO, 128, QH, 128).transpose(1, 2, 0, 3).copy()
    wk = Wk[:, kvcols].reshape(KO, 128, KVH, 128).transpose(1, 2, 0, 3).copy()
    wv = Wv[:, kvcols].reshape(KO, 128, KVH * 128).transpose(1, 0, 2).copy()
    wo = Wo[qcols, :].reshape(QH, 128, H).transpose(1, 0, 2).copy()

    pos = position_ids[bi].astype(f32)                              # [S]
    inv = (1.0 / (_ROPE_BASE ** (np.arange(0, _D, 2, dtype=f32) / _D)))  # [64]
    fr = pos[None, :] * inv[:, None]                                # [64, S]
    cos = np.concatenate([np.cos(fr), np.cos(fr)], axis=0).astype(f32)
    sin = np.concatenate([-np.sin(fr), np.sin(fr)], axis=0).astype(f32)

    inp = {"xt": np.ascontiguousarray(xt, f32),
           "wq": np.ascontiguousarray(wq, f32),
           "wk": np.ascontiguousarray(wk, f32),
           "wv": np.ascontiguousarray(wv, f32),
           "wo": np.ascontiguousarray(wo, f32),
           "cos": np.ascontiguousarray(cos, f32),
           "sin": np.ascontiguousarray(sin, f32)}
    if has_mask:
        mb = np.where(np.asarray(attn_mask[bi]) == 0, _MASK_NEG, 0.0)
        inp["maskb"] = np.ascontiguousarray(
            mb.reshape(S // 128, 128).T, f32)   # [128, KT], k = kt*128 + p
    return inp


def kernel(hidden_states, attn_mask, position_ids, Wq, bq, Wk, bk, Wv, bv,
           Wo, bo):
    from concourse.bass_utils import run_bass_kernel_spmd

    hidden_states = np.asarray(hidden_states, dtype=np.float32)
    attn_mask = np.asarray(attn_mask)
    position_ids = np.asarray(position_ids)
    Wq = np.asarray(Wq, dtype=np.float32)
    Wk = np.asarray(Wk, dtype=np.float32)
    Wv = np.asarray(Wv, dtype=np.float32)
    Wo = np.asarray(Wo, dtype=np.float32)
    bq = np.asarray(bq, dtype=np.float32)
    bk = np.asarray(bk, dtype=np.float32)
    bv = np.asarray(bv, dtype=np.float32)
    bo = np.asarray(bo, dtype=np.float32)

    has_mask = not bool(np.all(attn_mask == 1))
    # qkv biases are zero in this model config; fold nonzero ones into the
    # projection by augmenting is not implemented -- assert instead.
    assert not np.any(bq) and not np.any(bk) and not np.any(bv), \
        "nonzero qkv biases not supported"

    nc = _get_module(has_mask)
    in_maps = [make_core_inputs(hidden_states, attn_mask, position_ids,
                                Wq, Wk, Wv, Wo, c, has_mask)
               for c in range(_NCORES)]
    res = run_bass_kernel_spmd(nc, in_maps, core_ids=list(range(_NCORES)))

    out = np.zeros((_B, _S, _H), dtype=np.float32)
    for c in range(_NCORES):
        out[c // _TP] += res.results[c]["out"]
    out += bo[None, None, :]
    return out


# revision 4
# speedup vs baseline: 1.2735x; 1.2735x over previous
"""Llama attention (b=2, s=2048, h=4096, 32 q-heads / 8 kv-heads, d=128) on 8
Trainium2 NeuronCores.

Sharding: 2-way data parallel over batch x 4-way tensor parallel over head
groups. Core c handles batch c//4 and q-heads [8*(c%4), 8*(c%4)+8) (kv-heads
[2*(c%4), 2*(c%4)+2)). Each core computes a partial o_proj output (contraction
over its 1024 head dims); the host sums the 4 partials per batch and adds bo.

Device kernel (per core), all matmuls in float32r (fp22 mantissa, full PE rate):
  P1  QT/KT = W^T @ X^T accumulated over the 4096 contraction (RoPE fused into
      the PSUM eviction), V = X @ Wv in natural [s, d] layout (XT used as lhsT).
  P2  Scores are computed TRANSPOSED: ST[k, q] = K @ Q^T, so the mask bias and
      1/sqrt(32) scale fuse into the Exp activation (per-partition bias), the
      softmax needs no max subtraction (|scaled scores| << 80), no transposes
      are needed for P@V (lhsT=V[k,d], rhs=expST[k,q]), and the softmax
      denominator is a ones-lhsT matmul accumulated on the PE.
  P3  out = attn_out @ Wo_shard as a plain GEMM from the [d, head, q] staging.
"""

import math
import sys

sys.path.insert(0, "/opt/pypackages")
sys.path.insert(0, "/opt/trn_rl_repo")

import numpy as np

_H, _NH, _NKV, _D = 4096, 32, 8, 128
_B, _S = 2, 2048
_NCORES = 8
_TP = 4              # head-group parallel factor (x2 batch parallel)
_QH = _NH // _TP     # 8 q heads per core
_KVH = _NKV // _TP   # 2 kv heads per core
_G = _NH // _NKV     # 4 q heads per kv head
_SCALE = 1.0 / math.sqrt(float(_NH))
_ROPE_BASE = 10000.0
_MASK_NEG = -1.0e5   # exp(-1e5) == 0.0 in fp32; avoids inf/nan of finfo.min

_built = {}


def build_module(S, H, QH, KVH, G, has_mask):
    """Build the per-core Bass module. Returns the compiled Bacc object."""
    from concourse import bacc
    import concourse.mybir as mybir
    import concourse.tile as tile

    FP = mybir.dt.float32
    FPR = mybir.dt.float32r
    Exp = mybir.ActivationFunctionType.Copy  # placeholder; set below
    Exp = mybir.ActivationFunctionType.Exp
    Copy = mybir.ActivationFunctionType.Copy

    KO = H // 128          # contraction tiles for projections
    KO2 = KO // 2          # per K-half
    KT = S // 128          # key tiles for attention
    QBW = min(1024, S)     # P1 q block width
    NQB = S // QBW         # P1 q blocks
    NCK = QBW // 512       # 512-wide xt chunks per block
    NH1 = QH + 2 * KVH     # projection output heads (q, k, v)
    NQC = S // 512         # 512-wide q chunks in P2
    NT = H // 512          # 512-wide n chunks in P3
    NST = S // 128         # s tiles in P3

    nc = bacc.Bacc("TRN2", target_bir_lowering=False, debug=False,
                   num_devices=_NCORES)

    xt = nc.dram_tensor("xt", [128, 2, NQB * NCK, KO2, 512], FPR, kind="ExternalInput")
    wq = nc.dram_tensor("wq", [128, QH, 2, KO2, 128], FPR, kind="ExternalInput")
    wk = nc.dram_tensor("wk", [128, KVH, 2, KO2, 128], FPR, kind="ExternalInput")
    wv = nc.dram_tensor("wv", [128, KVH, 2, KO2, 128], FPR, kind="ExternalInput")
    wo = nc.dram_tensor("wo", [128, QH, H], FPR, kind="ExternalInput")
    cos = nc.dram_tensor("cos", [128, S], FP, kind="ExternalInput")
    sin = nc.dram_tensor("sin", [128, S], FP, kind="ExternalInput")
    if has_mask:
        maskb = nc.dram_tensor("maskb", [128, KT], FP, kind="ExternalInput")
    out = nc.dram_tensor("out", [S, H], FP, kind="ExternalOutput")

    def mm(ps, lhsT, rhs, start, stop):
        nc.tensor.matmul(ps, lhsT=lhsT, rhs=rhs, start=start, stop=stop)

    with tile.TileContext(nc) as tc:
        with tc.tile_pool(name="dram", bufs=1, space="DRAM") as dram:
            qt_d = dram.tile([QH, 128, S], FPR)
            kt_d = dram.tile([KVH, 128, S], FPR)
            v_d = dram.tile([KVH, S, 128], FPR)
            ot_d = dram.tile([QH, 128, S], FPR)

            # ---------------- P1: projections + RoPE ----------------
            # K-split GEMM: per 1024-wide q block, stream both K-halves of
            # XT once and all of Wq/Wk/Wv once (weights re-read only per
            # block), accumulating the half-K partials in SBUF. V is
            # projected in the same [d, s] T-layout and PE-transposed into
            # natural [s, d] tiles for P2's lhsT.
            from concourse.masks import make_identity
            with tc.tile_pool(name="p1c", bufs=1) as cpool, \
                 tc.tile_pool(name="p1r", bufs=1) as raw_pool, \
                 tc.tile_pool(name="p1xt", bufs=3) as xt_pool, \
                 tc.tile_pool(name="p1w", bufs=2) as w_pool, \
                 tc.tile_pool(name="p1ps", bufs=2, space="PSUM") as ps1, \
                 tc.tile_pool(name="p1pt", bufs=2, space="PSUM") as pst_pool, \
                 tc.tile_pool(name="p1t", bufs=2) as tpool, \
                 tc.tile_pool(name="p1o", bufs=2) as opool:

                cos_sb = cpool.tile([128, S], FP)
                sin_sb = cpool.tile([128, S], FP)
                ident = cpool.tile([128, 128], FP)
                nc.sync.dma_start(cos_sb[:], cos[:])
                nc.sync.dma_start(sin_sb[:], sin[:])
                make_identity(nc, ident)

                for qb in range(NQB):
                    raw = raw_pool.tile([128, NH1, QBW], FP, tag="raw")
                    for kh in range(2):
                        xt_c = []
                        for c in range(NCK):
                            t = xt_pool.tile([128, KO2, 512], FPR, tag="xt")
                            nc.sync.dma_start(t[:], xt[:, kh, qb * NCK + c])
                            xt_c.append(t)
                        for ih in range(NH1):
                            w_sb = w_pool.tile([128, KO2, 128], FPR, tag="w")
                            if ih < QH:
                                wsrc = wq[:, ih, kh]
                            elif ih < QH + KVH:
                                wsrc = wk[:, ih - QH, kh]
                            else:
                                wsrc = wv[:, ih - QH - KVH, kh]
                            nc.sync.dma_start(w_sb[:], wsrc)
                            ps = ps1.tile([128, QBW], FP)
                            for c in range(NCK):
                                for ko in range(KO2):
                                    mm(ps[:, c * 512:(c + 1) * 512],
                                       w_sb[:, ko], xt_c[c][:, ko],
                                       ko == 0, ko == KO2 - 1)
                            if kh == 0:
                                nc.vector.tensor_copy(raw[:, ih], ps[:])
                            else:
                                nc.vector.tensor_add(raw[:, ih], raw[:, ih],
                                                     ps[:])
                    cs = slice(qb * QBW, (qb + 1) * QBW)
                    for ih in range(QH + KVH):
                        # RoPE eviction (sin rows 0:64 = -sin, 64:128 = +sin)
                        t1 = tpool.tile([128, QBW], FP, tag="t1")
                        t2 = tpool.tile([128, QBW], FP, tag="t2")
                        ro = opool.tile([128, QBW], FPR, tag="ro")
                        nc.vector.tensor_mul(t1[:], raw[:, ih], cos_sb[:, cs])
                        nc.vector.tensor_mul(t2[0:64], raw[64:128, ih],
                                             sin_sb[0:64, cs])
                        nc.vector.tensor_mul(t2[64:128], raw[0:64, ih],
                                             sin_sb[64:128, cs])
                        nc.vector.tensor_add(ro[:], t1[:], t2[:])
                        dst = qt_d[ih, :, cs] if ih < QH else \
                            kt_d[ih - QH, :, cs]
                        nc.sync.dma_start(dst, ro[:])
                    for kv in range(KVH):
                        ih = QH + KVH + kv
                        for st in range(QBW // 128):
                            pt = pst_pool.tile([128, 128], FP)
                            nc.tensor.transpose(
                                pt[:], raw[:, ih, st * 128:(st + 1) * 128],
                                ident[:])
                            vo = opool.tile([128, 128], FPR, tag="vo")
                            nc.scalar.activation(vo[:], pt[:], Copy)
                            s0 = qb * QBW + st * 128
                            nc.sync.dma_start(v_d[kv, s0:s0 + 128, :], vo[:])

            # ---------------- P2: attention ----------------
            with tc.tile_pool(name="p2c", bufs=1) as cpool, \
                 tc.tile_pool(name="p2qt", bufs=2) as qt_pool, \
                 tc.tile_pool(name="p2e", bufs=6) as e_pool, \
                 tc.tile_pool(name="p2ot", bufs=2) as ot_pool, \
                 tc.tile_pool(name="p2sm", bufs=3) as sm_pool, \
                 tc.tile_pool(name="p2ps", bufs=3, space="PSUM") as s_psum, \
                 tc.tile_pool(name="p2po", bufs=2, space="PSUM") as o_psum, \
                 tc.tile_pool(name="p2pe", bufs=2, space="PSUM") as se_psum:

                kt_sb = cpool.tile([128, KVH, S], FPR)
                v_sb = cpool.tile([128, KVH, KT, 128], FPR)
                ones_f = cpool.tile([128, 1], FP)
                ones = cpool.tile([128, 1], FPR)
                nc.sync.dma_start(kt_sb[:], kt_d[:].rearrange("k p q -> p k q"))
                nc.sync.dma_start(
                    v_sb[:], v_d[:].rearrange("k (t p) d -> p k t d", p=128))
                nc.vector.memset(ones_f[:], 1.0)
                nc.vector.tensor_copy(ones[:], ones_f[:])
                if has_mask:
                    mb_sb = cpool.tile([128, KT], FP)
                    nc.sync.dma_start(mb_sb[:], maskb[:])

                for qc in range(NQC):
                    ccs = slice(qc * 512, qc * 512 + 512)
                    qt_sb = qt_pool.tile([128, QH, 512], FPR, tag="qt")
                    nc.sync.dma_start(
                        qt_sb[:], qt_d[:, :, ccs].rearrange("h p q -> p h q"))
                    ot_sb = ot_pool.tile([128, QH, 512], FPR, tag="ot")

                    for h in range(QH):
                        kv = h // G
                        o_ps = o_psum.tile([128, 512], FP)
                        se_ps = se_psum.tile([1, 512], FP)
                        for kt in range(KT):
                            s_ps = s_psum.tile([128, 512], FP)
                            mm(s_ps[:], kt_sb[:, kv, kt * 128:(kt + 1) * 128],
                               qt_sb[:, h], True, True)
                            e_sb = e_pool.tile([128, 512], FPR, tag="e")
                            if has_mask:
                                nc.scalar.activation(e_sb[:], s_ps[:], Exp,
                                                     bias=mb_sb[:, kt:kt + 1],
                                                     scale=_SCALE)
                            else:
                                nc.scalar.activation(e_sb[:], s_ps[:], Exp,
                                                     scale=_SCALE)
                            mm(o_ps[:], v_sb[:, kv, kt], e_sb[:],
                               kt == 0, kt == KT - 1)
                            mm(se_ps[:], ones[:], e_sb[:],
                               kt == 0, kt == KT - 1)
                        recip = sm_pool.tile([1, 512], FP, tag="rc")
                        nc.vector.reciprocal(recip[:], se_ps[:])
                        bc = sm_pool.tile([128, 512], FP, tag="bc")
                        nc.gpsimd.partition_broadcast(bc[:], recip[:])
                        nc.vector.tensor_mul(ot_sb[:, h], o_ps[:], bc[:])

                    nc.sync.dma_start(
                        ot_d[:, :, ccs].rearrange("h p q -> p h q"), ot_sb[:])

            # ---------------- P3: output projection ----------------
            with tc.tile_pool(name="p3c", bufs=1) as cpool, \
                 tc.tile_pool(name="p3w", bufs=2) as wo_pool, \
                 tc.tile_pool(name="p3o", bufs=3) as oo_pool, \
                 tc.tile_pool(name="p3ps", bufs=4, space="PSUM") as o3_psum:

                ot_full = cpool.tile([128, QH, S], FPR)
                nc.sync.dma_start(ot_full[:],
                                  ot_d[:].rearrange("h p q -> p h q"))
                for ncn in range(NT):
                    ncs = slice(ncn * 512, ncn * 512 + 512)
                    wo_sb = wo_pool.tile([128, QH, 512], FPR, tag="wo")
                    nc.sync.dma_start(wo_sb[:], wo[:, :, ncs])
                    for st in range(NST):
                        ps = o3_psum.tile([128, 512], FP)
                        for ho in range(QH):
                            mm(ps[:], ot_full[:, ho, st * 128:(st + 1) * 128],
                               wo_sb[:, ho], ho == 0, ho == QH - 1)
                        ob = oo_pool.tile([128, 512], FP, tag="ob")
                        nc.scalar.activation(ob[:], ps[:], Copy)
                        nc.sync.dma_start(out[st * 128:(st + 1) * 128, ncs],
                                          ob[:])

    nc.compile()
    return nc


def _get_module(has_mask):
    key = (_S, _H, _QH, _KVH, has_mask)
    if key not in _built:
        _built[key] = build_module(_S, _H, _QH, _KVH, _G, has_mask)
    return _built[key]


def make_core_inputs(hidden_states, attn_mask, position_ids, Wq, Wk, Wv, Wo,
                     core, has_mask, S=_S, H=_H, QH=_QH, KVH=_KVH):
    """Host-side shard + layout packing for one core."""
    bi, hg = core // _TP, core % _TP
    KO = H // 128
    KO2 = KO // 2
    NCH = S // 512
    f32 = np.float32

    XT = np.ascontiguousarray(hidden_states[bi].T.astype(f32))      # [H, S]
    # [128p, 2kh, NCH, KO2, 512]
    xt = XT.reshape(2, KO2, 128, NCH, 512).transpose(2, 0, 3, 1, 4).copy()

    def pack_w(W):  # [H, nh*128] -> [128p, nh, 2kh, KO2, 128]
        nh = W.shape[1] // 128
        return W.reshape(2, KO2, 128, nh, 128).transpose(2, 3, 0, 1, 4).copy()

    qcols = slice(hg * QH * 128, (hg + 1) * QH * 128)
    kvcols = slice(hg * KVH * 128, (hg + 1) * KVH * 128)
    wq = pack_w(Wq[:, qcols])
    wk = pack_w(Wk[:, kvcols])
    wv = pack_w(Wv[:, kvcols])
    wo = Wo[qcols, :].reshape(QH, 128, H).transpose(1, 0, 2).copy()

    pos = position_ids[bi].astype(f32)                              # [S]
    inv = (1.0 / (_ROPE_BASE ** (np.arange(0, _D, 2, dtype=f32) / _D)))  # [64]
    fr = pos[None, :] * inv[:, None]                                # [64, S]
    cos = np.concatenate([np.cos(fr), np.cos(fr)], axis=0).astype(f32)
    sin = np.concatenate([-np.sin(fr), np.sin(fr)], axis=0).astype(f32)

    inp = {"xt": np.ascontiguousarray(xt, f32),
           "wq": np.ascontiguousarray(wq, f32),
           "wk": np.ascontiguousarray(wk, f32),
           "wv": np.ascontiguousarray(wv, f32),
           "wo": np.ascontiguousarray(wo, f32),
           "cos": np.ascontiguousarray(cos, f32),
           "sin": np.ascontiguousarray(sin, f32)}
    if has_mask:
        mb = np.where(np.asarray(attn_mask[bi]) == 0, _MASK_NEG, 0.0)
        inp["maskb"] = np.ascontiguousarray(
            mb.reshape(S // 128, 128).T, f32)   # [128, KT], k = kt*128 + p
    return inp


def kernel(hidden_states, attn_mask, position_ids, Wq, bq, Wk, bk, Wv, bv,
           Wo, bo):
    from concourse.bass_utils import run_bass_kernel_spmd

    hidden_states = np.asarray(hidden_states, dtype=np.float32)
    attn_mask = np.asarray(attn_mask)
    position_ids = np.asarray(position_ids)
    Wq = np.asarray(Wq, dtype=np.float32)
    Wk = np.asarray(Wk, dtype=np.float32)
    Wv = np.asarray(Wv, dtype=np.float32)
    Wo = np.asarray(Wo, dtype=np.float32)
    bq = np.asarray(bq, dtype=np.float32)
    bk = np.asarray(bk, dtype=np.float32)
    bv = np.asarray(bv, dtype=np.float32)
    bo = np.asarray(bo, dtype=np.float32)

    has_mask = not bool(np.all(attn_mask == 1))
    # qkv biases are zero in this model config; fold nonzero ones into the
    # projection by augmenting is not implemented -- assert instead.
    assert not np.any(bq) and not np.any(bk) and not np.any(bv), \
        "nonzero qkv biases not supported"

    nc = _get_module(has_mask)
    in_maps = [make_core_inputs(hidden_states, attn_mask, position_ids,
                                Wq, Wk, Wv, Wo, c, has_mask)
               for c in range(_NCORES)]
    res = run_bass_kernel_spmd(nc, in_maps, core_ids=list(range(_NCORES)))

    out = np.zeros((_B, _S, _H), dtype=np.float32)
    for c in range(_NCORES):
        out[c // _TP] += res.results[c]["out"]
    out += bo[None, None, :]
    return out
